# revision 5
# baseline (speedup 1.0000x reference)
"""Trainium2 Bass kernel for nn_Direction: out = input @ qr(weight + 1e-8).Q.T

Strategy (data-parallel over 8 NeuronCores):
  - Host: Q = np.linalg.qr(weight + 1e-8).Q  (512x26, tiny; LAPACK Householder
    matches the jnp.linalg.qr sign convention). Replicated to all cores.
  - Host: shard input [262144, 26] by batch into 8 x [32768, 26], and hand each
    core its shard pre-transposed as [26, 32768] so the contraction dim (26) is
    the SBUF partition dim - the layout the PE matmul needs for lhsT.
  - Device (per core): for each 128-row batch tile j,
        psum[128, 512] = lhsT(xt[:, j*128:(j+1)*128]).T @ rhs(qt[26, 512])
    with float32r (fp32 bits, full-rate PE mode at N=512), copy PSUM->SBUF on
    DVE/ACT alternately (DMA cannot read PSUM on TRN2), and DMA 2 MiB staged
    chunks of the output back to HBM.
  - Host: concatenate the 8 x [32768, 512] shards.
"""

import sys

import numpy as np

try:
    import concourse  # noqa: F401
except ImportError:
    sys.path.insert(0, "/opt/trn_rl_repo")

from concourse import bacc, mybir, tile
from concourse.bass_utils import run_bass_kernel_spmd

N_CORES = 8
B = 262144
D = 26
OUT = 512
ROWS = B // N_CORES  # 32768 batch rows per core

MM = 128  # batch rows per matmul (PSUM partition dim)
STAGE = 8  # matmul tiles per staged output DMA (8 * 256 KiB = 2 MiB)
GROUP = STAGE * MM  # 1024 batch rows per staged output DMA
# Input DMA chunk sizes (batch rows). Graduated: small first chunks so the
# first matmuls (and therefore the output DMA stream) start ~5us earlier
# than with a uniform 8192 split; 26-partition input DMAs are port-limited.
CHUNKS = [1024, 3072, 4096, 8192, 8192, 8192]
assert sum(CHUNKS) == ROWS and all(c % GROUP == 0 for c in CHUNKS)

_F32 = mybir.dt.float32
_F32R = mybir.dt.float32r

_NC = None


def _emit(tc, xt, qt, out):
    nc = tc.nc
    with (
        tc.tile_pool(name="qt", bufs=1) as qt_pool,
        tc.tile_pool(name="xt", bufs=3) as xt_pool,
        tc.tile_pool(name="stage", bufs=4) as stage_pool,
        tc.tile_pool(name="psum", bufs=8, space="PSUM") as psum_pool,
    ):
        qt_sb = qt_pool.tile([D, OUT], _F32R)
        nc.sync.dma_start(qt_sb[:], qt[:])
        row = 0
        for chunk in CHUNKS:
            xt_sb = xt_pool.tile([D, max(CHUNKS)], _F32R, tag="xt_sb")
            nc.sync.dma_start(xt_sb[:, :chunk], xt[:, row : row + chunk])
            for g in range(chunk // GROUP):
                stage = stage_pool.tile([MM, STAGE * OUT], _F32)
                for t in range(STAGE):
                    j = g * STAGE + t
                    ps = psum_pool.tile([MM, OUT], _F32)
                    nc.tensor.matmul(
                        ps[:],
                        xt_sb[:, j * MM : (j + 1) * MM],
                        qt_sb[:],
                    )
                    dst = stage[:, t * OUT : (t + 1) * OUT]
                    if t % 2 == 0:
                        nc.vector.tensor_copy(dst, ps[:])
                    else:
                        nc.scalar.copy(dst, ps[:])
                base = row + g * GROUP
                out_view = out[base : base + GROUP, :].rearrange(
                    "(t p) o -> p t o", p=MM
                )
                stage_view = stage[:].rearrange("p (t o) -> p t o", t=STAGE)
                nc.sync.dma_start(out_view, stage_view)
            row += chunk


def _build():
    global _NC
    if _NC is not None:
        return _NC
    nc = bacc.Bacc(
        "TRN2", target_bir_lowering=False, debug=False, num_devices=N_CORES
    )
    xt = nc.dram_tensor("xt", [D, ROWS], _F32R, kind="ExternalInput").ap()
    qt = nc.dram_tensor("qt", [D, OUT], _F32R, kind="ExternalInput").ap()
    out = nc.dram_tensor("out", [ROWS, OUT], _F32, kind="ExternalOutput").ap()
    with tile.TileContext(nc) as tc:
        _emit(tc, xt, qt, out)
    nc.compile()
    _NC = nc
    return nc


def _run(in_maps, trace=False, **kwargs):
    nc = _build()
    return run_bass_kernel_spmd(
        nc, in_maps, list(range(N_CORES)), trace=trace, **kwargs
    )


def _prepare_in_maps(input, weight):
    x = np.asarray(input, dtype=np.float32)
    w = np.asarray(weight, dtype=np.float32)
    assert x.shape == (B, D) and w.shape == (OUT, D)
    q, _ = np.linalg.qr(w + np.float32(1e-8))
    qt = np.ascontiguousarray(q.T, dtype=np.float32)  # [26, 512]
    return [
        {
            "xt": np.ascontiguousarray(x[c * ROWS : (c + 1) * ROWS].T),
            "qt": qt,
        }
        for c in range(N_CORES)
    ]


def kernel(input, weight):
    in_maps = _prepare_in_maps(input, weight)
    res = _run(in_maps)
    return np.concatenate([r["out"] for r in res.results], axis=0)


# revision 6
# speedup vs baseline: 1.0714x; 1.0714x over previous
"""Trainium2 Bass kernel for nn_Direction: out = input @ qr(weight + 1e-8).Q.T

Strategy (data-parallel over 8 NeuronCores):
  - Host: Q = np.linalg.qr(weight + 1e-8).Q  (512x26, tiny; LAPACK Householder
    matches the jnp.linalg.qr sign convention). Replicated to all cores.
  - Host: shard input [262144, 26] by batch into 8 x [32768, 26], and hand each
    core its shard pre-transposed as [26, 32768] so the contraction dim (26) is
    the SBUF partition dim - the layout the PE matmul needs for lhsT.
  - Device (per core): for each 128-row batch tile j,
        psum[128, 512] = lhsT(xt[:, j*128:(j+1)*128]).T @ rhs(qt[26, 512])
    with float32r (fp32 bits, full-rate PE mode at N=512), copy PSUM->SBUF on
    DVE/ACT alternately (DMA cannot read PSUM on TRN2), and DMA 2 MiB staged
    chunks of the output back to HBM.
  - Host: concatenate the 8 x [32768, 512] shards.
"""

import sys

import numpy as np

try:
    import concourse  # noqa: F401
except ImportError:
    sys.path.insert(0, "/opt/trn_rl_repo")

from concourse import bacc, mybir, tile
from concourse.bass_utils import run_bass_kernel_spmd

N_CORES = 8
B = 262144
D = 26
OUT = 512
ROWS = B // N_CORES  # 32768 batch rows per core

MM = 128  # batch rows per matmul (PSUM partition dim)
STAGE = 8  # matmul tiles per staged output DMA (8 * 256 KiB = 2 MiB)
GROUP = STAGE * MM  # 1024 batch rows per staged output DMA
# Input DMA chunk sizes (batch rows). Graduated: small first chunks so the
# first matmuls (and therefore the output DMA stream) start ~5us earlier
# than with a uniform 8192 split; 26-partition input DMAs are port-limited.
CHUNKS = [1024, 3072, 4096, 8192, 8192, 8192]
assert sum(CHUNKS) == ROWS and all(c % GROUP == 0 for c in CHUNKS)

_F32 = mybir.dt.float32
_F32R = mybir.dt.float32r

_NC = None


def _emit(tc, xt, qt, out):
    nc = tc.nc
    with (
        tc.tile_pool(name="qt", bufs=1) as qt_pool,
        tc.tile_pool(name="xt", bufs=2) as xt_pool,
        tc.tile_pool(name="stage", bufs=4) as stage_pool,
        tc.tile_pool(name="psum", bufs=8, space="PSUM") as psum_pool,
    ):
        qt_sb = qt_pool.tile([D, OUT], _F32R)
        nc.scalar.dma_start(qt_sb[:], qt[:])
        row = 0
        for chunk in CHUNKS:
            xt_sb = xt_pool.tile([D, max(CHUNKS)], _F32R, tag="xt_sb")
            nc.scalar.dma_start(xt_sb[:, :chunk], xt[:, row : row + chunk])
            for g in range(chunk // GROUP):
                stage = stage_pool.tile([MM, STAGE * OUT], _F32)
                for t in range(STAGE):
                    j = g * STAGE + t
                    ps = psum_pool.tile([MM, OUT], _F32)
                    nc.tensor.matmul(
                        ps[:],
                        xt_sb[:, j * MM : (j + 1) * MM],
                        qt_sb[:],
                    )
                    dst = stage[:, t * OUT : (t + 1) * OUT]
                    if t % 2 == 0:
                        nc.vector.tensor_copy(dst, ps[:])
                    else:
                        nc.scalar.copy(dst, ps[:])
                base = row + g * GROUP
                out_view = out[base : base + GROUP, :].rearrange(
                    "(t p) o -> p t o", p=MM
                )
                stage_view = stage[:].rearrange("p (t o) -> p t o", t=STAGE)
                nc.sync.dma_start(out_view, stage_view)
            row += chunk


def _build():
    global _NC
    if _NC is not None:
        return _NC
    nc = bacc.Bacc(
        "TRN2", target_bir_lowering=False, debug=False, num_devices=N_CORES
    )
    xt = nc.dram_tensor("xt", [D, ROWS], _F32R, kind="ExternalInput").ap()
    qt = nc.dram_tensor("qt", [D, OUT], _F32R, kind="ExternalInput").ap()
    out = nc.dram_tensor("out", [ROWS, OUT], _F32, kind="ExternalOutput").ap()
    with tile.TileContext(nc) as tc:
        _emit(tc, xt, qt, out)
    nc.compile()
    _NC = nc
    return nc


def _run(in_maps, trace=False, **kwargs):
    nc = _build()
    return run_bass_kernel_spmd(
        nc, in_maps, list(range(N_CORES)), trace=trace, **kwargs
    )


def _prepare_in_maps(input, weight):
    x = np.asarray(input, dtype=np.float32)
    w = np.asarray(weight, dtype=np.float32)
    assert x.shape == (B, D) and w.shape == (OUT, D)
    q, _ = np.linalg.qr(w + np.float32(1e-8))
    qt = np.ascontiguousarray(q.T, dtype=np.float32)  # [26, 512]
    return [
        {
            "xt": np.ascontiguousarray(x[c * ROWS : (c + 1) * ROWS].T),
            "qt": qt,
        }
        for c in range(N_CORES)
    ]


def kernel(input, weight):
    in_maps = _prepare_in_maps(input, weight)
    res = _run(in_maps)
    return np.concatenate([r["out"] for r in res.results], axis=0)
